# revision 20
# baseline (speedup 1.0000x reference)
"""Multi-head latent attention (MLA) Bass kernel for 8 trn2 NeuronCores.

Sharding: batch (2) x head-groups (4 heads each) -> 8 cores. Each core
computes, for its batch b and its 4 heads:
  latent = x[b] @ wdkv.T          (replicated per batch-group)
  q = x[b] @ wq_heads.T, k = latent @ wuk_heads.T, v = latent @ wuv_heads.T
  RoPE(q, k); causal softmax attention; ctx
  partial out = ctx @ wo[:, head_cols].T
Host sums the 4 partials per batch and adds the bias.

All TensorE-facing data is bf16 (fp32 PSUM accumulation); rel err vs the
fp32 reference is ~1e-3..1e-2 scale, well inside the 2e-2 gate.

On-chip layouts (partition dim first):
  xT   [128 d, 16 dc, S]   (d-major so d is the contraction dim)
  qr/kr[128 hd, 4 h, S]    (RoPE'd, transposed)
  v    [128 s, 16 sc, 512 e]
  attnT[128 sk, 512 sq]    (scores computed transposed; row sums via
                            ones-matmul which also broadcasts across parts)
  ctxT [128 hd, 4 h, S]
  outT [o, s] in DRAM, host transposes back.
"""

import numpy as np
import ml_dtypes

import concourse.bass as bass
import concourse.tile as tile
import concourse.mybir as mybir

bf16 = ml_dtypes.bfloat16
BF16 = mybir.dt.bfloat16
FP32 = mybir.dt.float32
AF = mybir.ActivationFunctionType

B, S, D, L, H, hd = 2, 2048, 2048, 256, 16, 128
E = 512          # head dims per core (4 heads)
SQ = 512         # sq chunk (matmul free dim)
SK = 128         # sk chunk (partition dim)
NDC = D // 128   # 16 contraction chunks over d
NLC = L // 128   # 2 contraction chunks over latent
NEC = E // 128   # 4 e-chunks per core
NSQ = S // SQ    # 4
NSK = S // SK    # 16
SCALE = 1.0 / float(np.sqrt(hd))
ROPE_BASE = 10000.0

_CACHE = {}


def _split_waits(nc):
    """walrus in this container accepts at most ONE sync-wait per instruction
    (two on InstEventSemaphore), but the Tile scheduler emits more. Split the
    extras into preceding same-engine EventSemaphore carriers."""
    for bb in nc.m.functions[0].blocks:
        out, changed = [], False
        for ins in bb.instructions:
            si = ins.sync_info
            waits = list(si.on_wait) if si and si.on_wait else []
            cap = 2 if isinstance(ins, mybir.InstEventSemaphore) else 1
            if len(waits) > cap:
                changed = True
                extra, keep = waits[:-cap], waits[-cap:]
                for i in range(0, len(extra), 2):
                    out.append(mybir.InstEventSemaphore(
                        name=f"{ins.name}-evw{i}",
                        engine=ins.engine, ins=[], outs=[],
                        sync_info=mybir.SyncInfo(
                            on_wait=extra[i:i + 2], on_update=[]),
                    ))
                ins.sync_info = mybir.SyncInfo(
                    on_wait=keep,
                    on_update=list(si.on_update) if si and si.on_update else [])
            out.append(ins)
        if changed:
            bb.instructions = out


def _build(split=True):
    nc = bass.Bass()
    xT = nc.dram_tensor("xT", [D, S], BF16, kind="ExternalInput")
    wqT = nc.dram_tensor("wqT", [D, E], BF16, kind="ExternalInput")
    wdkvT = nc.dram_tensor("wdkvT", [D, L], BF16, kind="ExternalInput")
    wukT = nc.dram_tensor("wukT", [L, E], BF16, kind="ExternalInput")
    wuvT = nc.dram_tensor("wuvT", [L, E], BF16, kind="ExternalInput")
    woT = nc.dram_tensor("woT", [E, D], BF16, kind="ExternalInput")
    cosT = nc.dram_tensor("cosT", [hd, S], FP32, kind="ExternalInput")
    sinrot = nc.dram_tensor("sinrot", [hd, S], FP32, kind="ExternalInput")
    masks = nc.dram_tensor("masks", [4, SK, SQ], BF16, kind="ExternalInput")
    outT = nc.dram_tensor("outT", [D, S], FP32, kind="ExternalOutput")

    xT_r = xT[:].rearrange("(dc p) s -> p dc s", p=128)
    wqT_r = wqT[:].rearrange("(dc p) e -> p dc e", p=128)
    wdkvT_r = wdkvT[:].rearrange("(dc p) l -> p dc l", p=128)
    wukT_r = wukT[:].rearrange("(lc p) e -> p lc e", p=128)
    wuvT_r = wuvT[:].rearrange("(lc p) e -> p lc e", p=128)
    woT_r = woT[:].rearrange("(ec p) o -> p ec o", p=128)
    masks_r = masks[:].rearrange("r p f -> p r f")
    outT_r = outT[:].rearrange("(oc p) s -> p oc s", p=128)

    with tile.TileContext(nc) as tc:
        with (
            tc.tile_pool(name="singles", bufs=1) as singles,
            tc.tile_pool(name="persist", bufs=1) as persist,
            tc.tile_pool(name="xpool", bufs=2) as xpool,
            tc.tile_pool(name="latp", bufs=2) as latp,
            tc.tile_pool(name="attp", bufs=4) as attp,
            tc.tile_pool(name="small", bufs=2) as small,
            tc.tile_pool(name="opool", bufs=3) as opool,
            tc.tile_pool(name="psum", bufs=3, space="PSUM") as psum,
            tc.tile_pool(name="psum_acc", bufs=1, space="PSUM") as psum_acc,
        ):
            # ---- weights / tables (scalar HWDGE queue, so the x stream on
            # the sync queue isn't stuck behind ~12MB of weights) ----
            wdkv_sb = singles.tile([128, NDC, L], BF16)
            nc.scalar.dma_start(wdkv_sb, wdkvT_r)
            wq_sb = singles.tile([128, NDC, E], BF16)
            nc.scalar.dma_start(wq_sb, wqT_r)
            wuk_sb = singles.tile([128, NLC, E], BF16)
            nc.scalar.dma_start(wuk_sb, wukT_r)
            wuv_sb = singles.tile([128, NLC, E], BF16)
            nc.scalar.dma_start(wuv_sb, wuvT_r)
            cos_sb = singles.tile([128, S], FP32)
            nc.scalar.dma_start(cos_sb, cosT[:])
            sr_sb = singles.tile([128, S], FP32)
            nc.scalar.dma_start(sr_sb, sinrot[:])
            mask_sb = singles.tile([128, 4, SQ], BF16)
            nc.scalar.dma_start(mask_sb, masks_r)
            wo_sb = singles.tile([128, NEC, D], BF16)
            nc.scalar.dma_start(wo_sb, woT_r)
            ones_sb = singles.tile([128, 128], BF16)
            nc.vector.memset(ones_sb, 1.0)

            qr = persist.tile([128, NEC, S], BF16)
            kr = persist.tile([128, NEC, S], BF16)
            vv = persist.tile([128, NSK, E], BF16)
            ctxT = persist.tile([128, NEC, S], BF16)

            def rope(dst, src_ps, jsl):
                # dst[p, s] = src[p, s]*cos[p, s] + src[(p+64)%128, s]*sinrot[p, s]
                tmp = small.tile([128, SQ], FP32, tag="ropetmp")
                nc.vector.tensor_mul(tmp[0:64], src_ps[64:128], sr_sb[0:64, jsl])
                nc.vector.tensor_mul(tmp[64:128], src_ps[0:64], sr_sb[64:128, jsl])
                nc.vector.tensor_mul(dst, src_ps, cos_sb[:, jsl])
                nc.vector.tensor_add(dst, dst, tmp)

            # ---- phase 1: projections ----
            for j in range(NSQ):
                jsl = bass.ts(j, SQ)
                x_sb = xpool.tile([128, NDC, SQ], BF16)
                for g in range(0, NDC, 4):  # split DMA so matmuls start early
                    nc.sync.dma_start(
                        x_sb[:, g:g + 4, :], xT_r[:, g:g + 4, jsl])

                lat = latp.tile([128, NLC, SQ], BF16)
                lp = psum.tile([128, 2, SQ], FP32, tag="pair")
                for lc in range(NLC):
                    for dc in range(NDC):
                        nc.tensor.matmul(
                            lp[:, lc, :], wdkv_sb[:, dc, bass.ts(lc, 128)],
                            x_sb[:, dc, :],
                            start=(dc == 0), stop=(dc == NDC - 1))
                nc.scalar.copy(lat, lp)

                # thin PE stages (v, k) first; the heavy q stage then overlaps
                # the DVE rope drain so pair-psum slots recycle in time
                for sp2 in range(2):
                    vp = psum.tile([128, 2, SQ], FP32, tag="pair")
                    for d in range(2):
                        sc = 2 * sp2 + d
                        for lc in range(NLC):
                            nc.tensor.matmul(
                                vp[:, d, :], lat[:, lc, bass.ts(sc, 128)],
                                wuv_sb[:, lc, :],
                                start=(lc == 0), stop=(lc == NLC - 1))
                    nc.scalar.copy(vv[:, 4 * j + 2 * sp2: 4 * j + 2 * sp2 + 2, :], vp)

                for ep in range(NEC // 2):
                    kp = psum.tile([128, 2, SQ], FP32, tag="pair")
                    for d in range(2):
                        ec = 2 * ep + d
                        for lc in range(NLC):
                            nc.tensor.matmul(
                                kp[:, d, :], wuk_sb[:, lc, bass.ts(ec, 128)],
                                lat[:, lc, :],
                                start=(lc == 0), stop=(lc == NLC - 1))
                    rope(kr[:, 2 * ep, jsl], kp[:, 0, :], jsl)
                    rope(kr[:, 2 * ep + 1, jsl], kp[:, 1, :], jsl)

                for ep in range(NEC // 2):
                    qp = psum.tile([128, 2, SQ], FP32, tag="pair")
                    for d in range(2):
                        ec = 2 * ep + d
                        for dc in range(NDC):
                            nc.tensor.matmul(
                                qp[:, d, :], wq_sb[:, dc, bass.ts(ec, 128)],
                                x_sb[:, dc, :],
                                start=(dc == 0), stop=(dc == NDC - 1))
                    rope(qr[:, 2 * ep, jsl], qp[:, 0, :], jsl)
                    rope(qr[:, 2 * ep + 1, jsl], qp[:, 1, :], jsl)

            # ---- phase 2: attention ----
            for j in range(NSQ):
                jsl = bass.ts(j, SQ)
                n_i = 4 * j + 4
                for h in range(NEC):
                    sums_ps = psum_acc.tile([128, SQ], FP32, tag="sums")
                    ctx_ps = psum_acc.tile([128, SQ], FP32, tag="ctx")
                    for i2 in range(0, n_i, 2):
                        sp = psum.tile([128, 2, SQ], FP32, tag="pair")
                        for d in range(2):
                            nc.tensor.matmul(
                                sp[:, d, :], kr[:, h, bass.ts(i2 + d, 128)],
                                qr[:, h, jsl], start=True, stop=True)
                        att = attp.tile([128, 2, SQ], BF16)
                        nc.scalar.activation(att, sp, AF.Exp, scale=SCALE)
                        if i2 >= 4 * j:  # diagonal region (pairs never straddle)
                            rr = i2 - 4 * j
                            nc.gpsimd.tensor_mul(att, att, mask_sb[:, rr:rr + 2, :])
                        for d in range(2):
                            i = i2 + d
                            nc.tensor.matmul(
                                sums_ps, ones_sb, att[:, d, :],
                                start=(i == 0), stop=(i == n_i - 1))
                            nc.tensor.matmul(
                                ctx_ps, vv[:, i, bass.ts(h, 128)], att[:, d, :],
                                start=(i == 0), stop=(i == n_i - 1))
                    recip = small.tile([128, SQ], FP32, tag="recip")
                    nc.vector.reciprocal(recip, sums_ps)
                    nc.vector.tensor_mul(ctxT[:, h, jsl], ctx_ps, recip)

            # ---- phase 3: output projection ----
            for j in range(NSQ):
                jsl = bass.ts(j, SQ)
                for op2 in range(NDC // 2):
                    op = psum.tile([128, 2, SQ], FP32, tag="pair")
                    for d in range(2):
                        oc = 2 * op2 + d
                        for ec in range(NEC):
                            nc.tensor.matmul(
                                op[:, d, :], wo_sb[:, ec, bass.ts(oc, 128)],
                                ctxT[:, ec, jsl],
                                start=(ec == 0), stop=(ec == NEC - 1))
                    o_sb = opool.tile([128, 2, SQ], FP32)
                    if op2 % 2 == 0:
                        nc.scalar.copy(o_sb, op)
                    else:
                        nc.vector.tensor_copy(o_sb, op)
                    nc.scalar.dma_start(
                        outT_r[:, 2 * op2:2 * op2 + 2, jsl], o_sb)

    if split:
        _split_waits(nc)
    return nc


def _host_tables():
    inv_freq = 1.0 / (ROPE_BASE ** (np.arange(0, hd, 2, dtype=np.float32) / hd))
    freqs = np.arange(S, dtype=np.float32)[:, None] * inv_freq[None, :]
    emb = np.concatenate([freqs, freqs], axis=-1)          # [S, hd]
    cosT = np.ascontiguousarray(np.cos(emb).T.astype(np.float32))  # [hd, S]
    sinT = np.ascontiguousarray(np.sin(emb).T.astype(np.float32))
    sinrot = np.concatenate([-sinT[:64], sinT[64:]], axis=0)
    masks = np.zeros((4, SK, SQ), bf16)
    pp, ff = np.meshgrid(np.arange(SK), np.arange(SQ), indexing="ij")
    for r in range(4):
        masks[r] = (128 * r + pp <= ff).astype(bf16)
    return cosT, np.ascontiguousarray(sinrot), masks


def _make_in_maps(x, wq, wdkv, wuk, wuv, wo):
    cosT, sinrot, masks = _host_tables()
    wdkvT = np.ascontiguousarray(wdkv.T).astype(bf16)
    in_maps = []
    for c in range(8):
        b, hg = c // 4, c % 4
        es = slice(E * hg, E * hg + E)
        in_maps.append({
            "xT": np.ascontiguousarray(x[b].T).astype(bf16),
            "wqT": np.ascontiguousarray(wq[es].T).astype(bf16),
            "wdkvT": wdkvT,
            "wukT": np.ascontiguousarray(wuk[es].T).astype(bf16),
            "wuvT": np.ascontiguousarray(wuv[es].T).astype(bf16),
            "woT": np.ascontiguousarray(wo[:, es].T).astype(bf16),
            "cosT": cosT,
            "sinrot": sinrot,
            "masks": masks,
        })
    return in_maps


def _get_runner():
    """Build the Bass module once and wrap it in a non-donating jitted
    shard_map over 8 cores. Returns (run_fn, in_names) where run_fn takes
    concatenated [8*dim0, ...] arrays (or resident jax arrays) for each
    input and returns the list of concatenated outputs."""
    if "runner" in _CACHE:
        return _CACHE["runner"]
    import jax
    from jax.sharding import Mesh, PartitionSpec as P
    from jax.experimental.shard_map import shard_map
    from concourse.bass2jax import (
        _bass_exec_p, install_neuronx_cc_hook, partition_id_tensor)

    install_neuronx_cc_hook()
    nc = _CACHE.setdefault("nc", _build())

    pname = nc.partition_id_tensor.name if nc.partition_id_tensor else None
    in_names, out_names, out_avals = [], [], []
    for alloc in nc.m.functions[0].allocations:
        if not isinstance(alloc, mybir.MemoryLocationSet):
            continue
        name = alloc.memorylocations[0].name
        if alloc.kind == "ExternalInput":
            if name != pname:
                in_names.append(name)
        elif alloc.kind == "ExternalOutput":
            out_names.append(name)
            out_avals.append(jax.core.ShapedArray(
                tuple(alloc.tensor_shape), mybir.dt.np(alloc.dtype)))
    n_params, n_outs = len(in_names), len(out_names)
    bind_kw = dict(
        out_avals=tuple(out_avals),
        in_names=tuple(in_names + out_names + ([pname] if pname else [])),
        out_names=tuple(out_names),
        lowering_input_output_aliases=(),
        sim_require_finite=True,
        sim_require_nnan=True,
        nc=nc,
    )

    def _body(*args):
        ops = list(args)
        if pname:
            ops.append(partition_id_tensor())
        return tuple(_bass_exec_p.bind(*ops, **bind_kw))

    devices = jax.devices()[:8]
    mesh = Mesh(np.asarray(devices), ("core",))
    fn = jax.jit(
        shard_map(_body, mesh=mesh,
                  in_specs=(P("core"),) * (n_params + n_outs),
                  out_specs=(P("core"),) * n_outs, check_rep=False),
        keep_unused=True)
    zero_shapes = [(8 * a.shape[0], *a.shape[1:]) for a in out_avals]
    zero_dtypes = [a.dtype for a in out_avals]
    _CACHE["runner"] = (fn, in_names, out_names, zero_shapes, zero_dtypes, mesh)
    return _CACHE["runner"]


def kernel(x, wq, wdkv, wuk, wuv, wo, bo):
    x = np.asarray(x, np.float32)
    wq = np.asarray(wq, np.float32)
    wdkv = np.asarray(wdkv, np.float32)
    wuk = np.asarray(wuk, np.float32)
    wuv = np.asarray(wuv, np.float32)
    wo = np.asarray(wo, np.float32)
    bo = np.asarray(bo, np.float32)

    fn, in_names, out_names, zero_shapes, zero_dtypes, _ = _get_runner()
    in_maps = _make_in_maps(x, wq, wdkv, wuk, wuv, wo)
    args = [
        np.concatenate([np.asarray(in_maps[c][n]) for c in range(8)], axis=0)
        for n in in_names
    ] + [np.zeros(sh, dt) for sh, dt in zip(zero_shapes, zero_dtypes)]
    outs = fn(*args)
    outT = np.asarray(outs[out_names.index("outT")]).reshape(8, D, S)
    out = np.zeros((B, S, D), np.float32)
    for c in range(8):
        out[c // 4] += outT[c].T
    out += bo[None, None, :]
    return out


# revision 21
# speedup vs baseline: 1.1459x; 1.1459x over previous
"""Multi-head latent attention (MLA) Bass kernel for 8 trn2 NeuronCores.

Sharding: batch (2) x head-groups (4 heads each) -> 8 cores. Each core
computes, for its batch b and its 4 heads:
  latent = x[b] @ wdkv.T          (replicated per batch-group)
  q = x[b] @ wq_heads.T, k = latent @ wuk_heads.T, v = latent @ wuv_heads.T
  RoPE(q, k); causal softmax attention; ctx
  partial out = ctx @ wo[:, head_cols].T
Host sums the 4 partials per batch and adds the bias.

All TensorE-facing data is bf16 (fp32 PSUM accumulation); rel err vs the
fp32 reference is ~1e-3..1e-2 scale, well inside the 2e-2 gate.

On-chip layouts (partition dim first):
  xT   [128 d, 16 dc, S]   (d-major so d is the contraction dim)
  qr/kr[128 hd, 4 h, S]    (RoPE'd, transposed)
  v    [128 s, 16 sc, 512 e]
  attnT[128 sk, 512 sq]    (scores computed transposed; row sums via
                            ones-matmul which also broadcasts across parts)
  ctxT [128 hd, 4 h, S]
  outT [o, s] in DRAM, host transposes back.
"""

import numpy as np
import ml_dtypes

import concourse.bass as bass
import concourse.tile as tile
import concourse.mybir as mybir

bf16 = ml_dtypes.bfloat16
BF16 = mybir.dt.bfloat16
FP32 = mybir.dt.float32
AF = mybir.ActivationFunctionType

B, S, D, L, H, hd = 2, 2048, 2048, 256, 16, 128
E = 512          # head dims per core (4 heads)
SQ = 512         # sq chunk (matmul free dim)
SK = 128         # sk chunk (partition dim)
NDC = D // 128   # 16 contraction chunks over d
NLC = L // 128   # 2 contraction chunks over latent
NEC = E // 128   # 4 e-chunks per core
NSQ = S // SQ    # 4
NSK = S // SK    # 16
SCALE = 1.0 / float(np.sqrt(hd))
ROPE_BASE = 10000.0

_CACHE = {}


def _split_waits(nc):
    """walrus in this container accepts at most ONE sync-wait per instruction
    (two on InstEventSemaphore), but the Tile scheduler emits more. Split the
    extras into preceding same-engine EventSemaphore carriers."""
    for bb in nc.m.functions[0].blocks:
        out, changed = [], False
        for ins in bb.instructions:
            si = ins.sync_info
            waits = list(si.on_wait) if si and si.on_wait else []
            cap = 2 if isinstance(ins, mybir.InstEventSemaphore) else 1
            if len(waits) > cap:
                changed = True
                extra, keep = waits[:-cap], waits[-cap:]
                for i in range(0, len(extra), 2):
                    out.append(mybir.InstEventSemaphore(
                        name=f"{ins.name}-evw{i}",
                        engine=ins.engine, ins=[], outs=[],
                        sync_info=mybir.SyncInfo(
                            on_wait=extra[i:i + 2], on_update=[]),
                    ))
                ins.sync_info = mybir.SyncInfo(
                    on_wait=keep,
                    on_update=list(si.on_update) if si and si.on_update else [])
            out.append(ins)
        if changed:
            bb.instructions = out


def _build(split=True):
    nc = bass.Bass()
    xT = nc.dram_tensor("xT", [D, S], BF16, kind="ExternalInput")
    wqT = nc.dram_tensor("wqT", [D, E], BF16, kind="ExternalInput")
    wdkvT = nc.dram_tensor("wdkvT", [D, L], BF16, kind="ExternalInput")
    wukT = nc.dram_tensor("wukT", [L, E], BF16, kind="ExternalInput")
    wuvT = nc.dram_tensor("wuvT", [L, E], BF16, kind="ExternalInput")
    woT = nc.dram_tensor("woT", [E, D], BF16, kind="ExternalInput")
    cosT = nc.dram_tensor("cosT", [hd, S], FP32, kind="ExternalInput")
    sinrot = nc.dram_tensor("sinrot", [hd, S], FP32, kind="ExternalInput")
    masks = nc.dram_tensor("masks", [4, SK, SQ], BF16, kind="ExternalInput")
    outT = nc.dram_tensor("outT", [D, S], FP32, kind="ExternalOutput")

    xT_r = xT[:].rearrange("(dc p) s -> p dc s", p=128)
    wqT_r = wqT[:].rearrange("(dc p) e -> p dc e", p=128)
    wdkvT_r = wdkvT[:].rearrange("(dc p) l -> p dc l", p=128)
    wukT_r = wukT[:].rearrange("(lc p) e -> p lc e", p=128)
    wuvT_r = wuvT[:].rearrange("(lc p) e -> p lc e", p=128)
    woT_r = woT[:].rearrange("(ec p) o -> p ec o", p=128)
    masks_r = masks[:].rearrange("r p f -> p r f")
    outT_r = outT[:].rearrange("(oc p) s -> p oc s", p=128)

    with tile.TileContext(nc) as tc:
        with (
            tc.tile_pool(name="singles", bufs=1) as singles,
            tc.tile_pool(name="persist", bufs=1) as persist,
            tc.tile_pool(name="xpool", bufs=2) as xpool,
            tc.tile_pool(name="latp", bufs=3) as latp,
            tc.tile_pool(name="attp", bufs=6) as attp,
            tc.tile_pool(name="small", bufs=2) as small,
            tc.tile_pool(name="opool", bufs=2) as opool,
            tc.tile_pool(name="psum", bufs=3, space="PSUM") as psum,
            tc.tile_pool(name="psum_acc", bufs=1, space="PSUM") as psum_acc,
        ):
            # ---- weights / tables (scalar HWDGE queue, so the x stream on
            # the sync queue isn't stuck behind ~12MB of weights) ----
            wdkv_sb = singles.tile([128, NDC, L], BF16)
            nc.scalar.dma_start(wdkv_sb, wdkvT_r)
            wq_sb = singles.tile([128, NDC, E], BF16)
            nc.scalar.dma_start(wq_sb, wqT_r)
            wuk_sb = singles.tile([128, NLC, E], BF16)
            nc.scalar.dma_start(wuk_sb, wukT_r)
            wuv_sb = singles.tile([128, NLC, E], BF16)
            nc.scalar.dma_start(wuv_sb, wuvT_r)
            cos_sb = singles.tile([128, S], FP32)
            nc.scalar.dma_start(cos_sb, cosT[:])
            sr_sb = singles.tile([128, S], FP32)
            nc.scalar.dma_start(sr_sb, sinrot[:])
            mask_sb = singles.tile([128, 4, SQ], BF16)
            nc.scalar.dma_start(mask_sb, masks_r)
            wo_sb = singles.tile([128, NEC, D], BF16)
            nc.scalar.dma_start(wo_sb, woT_r)
            ones_sb = singles.tile([128, 128], BF16)
            nc.vector.memset(ones_sb, 1.0)

            qr = persist.tile([128, NEC, S], BF16)
            kr = persist.tile([128, NEC, S], BF16)
            vv = persist.tile([128, NSK, E], BF16)
            ctxT = persist.tile([128, NEC, S], BF16)

            def rope(dst, src_ps, jsl):
                # dst[p, s] = src[p, s]*cos[p, s] + src[(p+64)%128, s]*sinrot[p, s]
                tmp = small.tile([128, SQ], FP32, tag="ropetmp")
                nc.vector.tensor_mul(tmp[0:64], src_ps[64:128], sr_sb[0:64, jsl])
                nc.vector.tensor_mul(tmp[64:128], src_ps[0:64], sr_sb[64:128, jsl])
                nc.vector.tensor_mul(dst, src_ps, cos_sb[:, jsl])
                nc.vector.tensor_add(dst, dst, tmp)

            # ---- phase 1: projections ----
            for j in range(NSQ):
                jsl = bass.ts(j, SQ)
                x_sb = xpool.tile([128, NDC, SQ], BF16)
                for g in range(0, NDC, 4):  # split DMA so matmuls start early
                    nc.sync.dma_start(
                        x_sb[:, g:g + 4, :], xT_r[:, g:g + 4, jsl])

                lat = latp.tile([128, NLC, SQ], BF16)
                lp = psum.tile([128, 2, SQ], FP32, tag="pair")
                for lc in range(NLC):
                    for dc in range(NDC):
                        nc.tensor.matmul(
                            lp[:, lc, :], wdkv_sb[:, dc, bass.ts(lc, 128)],
                            x_sb[:, dc, :],
                            start=(dc == 0), stop=(dc == NDC - 1))
                nc.scalar.copy(lat, lp)

                # thin PE stages (v, k) first; the heavy q stage then overlaps
                # the DVE rope drain so pair-psum slots recycle in time
                for sp2 in range(2):
                    vp = psum.tile([128, 2, SQ], FP32, tag="pair")
                    for d in range(2):
                        sc = 2 * sp2 + d
                        for lc in range(NLC):
                            nc.tensor.matmul(
                                vp[:, d, :], lat[:, lc, bass.ts(sc, 128)],
                                wuv_sb[:, lc, :],
                                start=(lc == 0), stop=(lc == NLC - 1))
                    nc.scalar.copy(vv[:, 4 * j + 2 * sp2: 4 * j + 2 * sp2 + 2, :], vp)

                for ep in range(NEC // 2):
                    kp = psum.tile([128, 2, SQ], FP32, tag="pair")
                    for d in range(2):
                        ec = 2 * ep + d
                        for lc in range(NLC):
                            nc.tensor.matmul(
                                kp[:, d, :], wuk_sb[:, lc, bass.ts(ec, 128)],
                                lat[:, lc, :],
                                start=(lc == 0), stop=(lc == NLC - 1))
                    rope(kr[:, 2 * ep, jsl], kp[:, 0, :], jsl)
                    rope(kr[:, 2 * ep + 1, jsl], kp[:, 1, :], jsl)

                for ep in range(NEC // 2):
                    qp = psum.tile([128, 2, SQ], FP32, tag="pair")
                    for d in range(2):
                        ec = 2 * ep + d
                        for dc in range(NDC):
                            nc.tensor.matmul(
                                qp[:, d, :], wq_sb[:, dc, bass.ts(ec, 128)],
                                x_sb[:, dc, :],
                                start=(dc == 0), stop=(dc == NDC - 1))
                    rope(qr[:, 2 * ep, jsl], qp[:, 0, :], jsl)
                    rope(qr[:, 2 * ep + 1, jsl], qp[:, 1, :], jsl)

            # ---- phase 2: attention ----
            for j in range(NSQ):
                jsl = bass.ts(j, SQ)
                n_i = 4 * j + 4
                for h in range(NEC):
                    sums_ps = psum_acc.tile([128, SQ], FP32, tag="sums")
                    ctx_ps = psum_acc.tile([128, SQ], FP32, tag="ctx")
                    for i2 in range(0, n_i, 2):
                        sp = psum.tile([128, 2, SQ], FP32, tag="pair")
                        for d in range(2):
                            nc.tensor.matmul(
                                sp[:, d, :], kr[:, h, bass.ts(i2 + d, 128)],
                                qr[:, h, jsl], start=True, stop=True)
                        att = attp.tile([128, 2, SQ], BF16)
                        nc.scalar.activation(att, sp, AF.Exp, scale=SCALE)
                        if i2 >= 4 * j:  # diagonal region (pairs never straddle)
                            rr = i2 - 4 * j
                            nc.gpsimd.tensor_mul(att, att, mask_sb[:, rr:rr + 2, :])
                        for d in range(2):
                            i = i2 + d
                            nc.tensor.matmul(
                                sums_ps, ones_sb, att[:, d, :],
                                start=(i == 0), stop=(i == n_i - 1))
                            nc.tensor.matmul(
                                ctx_ps, vv[:, i, bass.ts(h, 128)], att[:, d, :],
                                start=(i == 0), stop=(i == n_i - 1))
                    recip = small.tile([128, SQ], FP32, tag="recip")
                    nc.vector.reciprocal(recip, sums_ps)
                    nc.vector.tensor_mul(ctxT[:, h, jsl], ctx_ps, recip)

            # ---- phase 3: output projection ----
            for j in range(NSQ):
                jsl = bass.ts(j, SQ)
                for op2 in range(NDC // 2):
                    op = psum.tile([128, 2, SQ], FP32, tag="pair")
                    for d in range(2):
                        oc = 2 * op2 + d
                        for ec in range(NEC):
                            nc.tensor.matmul(
                                op[:, d, :], wo_sb[:, ec, bass.ts(oc, 128)],
                                ctxT[:, ec, jsl],
                                start=(ec == 0), stop=(ec == NEC - 1))
                    o_sb = opool.tile([128, 2, SQ], FP32)
                    if op2 % 2 == 0:
                        nc.scalar.copy(o_sb, op)
                    else:
                        nc.vector.tensor_copy(o_sb, op)
                    nc.scalar.dma_start(
                        outT_r[:, 2 * op2:2 * op2 + 2, jsl], o_sb)

    if split:
        _split_waits(nc)
    return nc


def _host_tables():
    inv_freq = 1.0 / (ROPE_BASE ** (np.arange(0, hd, 2, dtype=np.float32) / hd))
    freqs = np.arange(S, dtype=np.float32)[:, None] * inv_freq[None, :]
    emb = np.concatenate([freqs, freqs], axis=-1)          # [S, hd]
    cosT = np.ascontiguousarray(np.cos(emb).T.astype(np.float32))  # [hd, S]
    sinT = np.ascontiguousarray(np.sin(emb).T.astype(np.float32))
    sinrot = np.concatenate([-sinT[:64], sinT[64:]], axis=0)
    masks = np.zeros((4, SK, SQ), bf16)
    pp, ff = np.meshgrid(np.arange(SK), np.arange(SQ), indexing="ij")
    for r in range(4):
        masks[r] = (128 * r + pp <= ff).astype(bf16)
    return cosT, np.ascontiguousarray(sinrot), masks


def _make_in_maps(x, wq, wdkv, wuk, wuv, wo):
    cosT, sinrot, masks = _host_tables()
    wdkvT = np.ascontiguousarray(wdkv.T).astype(bf16)
    in_maps = []
    for c in range(8):
        b, hg = c // 4, c % 4
        es = slice(E * hg, E * hg + E)
        in_maps.append({
            "xT": np.ascontiguousarray(x[b].T).astype(bf16),
            "wqT": np.ascontiguousarray(wq[es].T).astype(bf16),
            "wdkvT": wdkvT,
            "wukT": np.ascontiguousarray(wuk[es].T).astype(bf16),
            "wuvT": np.ascontiguousarray(wuv[es].T).astype(bf16),
            "woT": np.ascontiguousarray(wo[:, es].T).astype(bf16),
            "cosT": cosT,
            "sinrot": sinrot,
            "masks": masks,
        })
    return in_maps


def _get_runner():
    """Build the Bass module once and wrap it in a non-donating jitted
    shard_map over 8 cores. Returns (run_fn, in_names) where run_fn takes
    concatenated [8*dim0, ...] arrays (or resident jax arrays) for each
    input and returns the list of concatenated outputs."""
    if "runner" in _CACHE:
        return _CACHE["runner"]
    import jax
    from jax.sharding import Mesh, PartitionSpec as P
    from jax.experimental.shard_map import shard_map
    from concourse.bass2jax import (
        _bass_exec_p, install_neuronx_cc_hook, partition_id_tensor)

    install_neuronx_cc_hook()
    nc = _CACHE.setdefault("nc", _build())

    pname = nc.partition_id_tensor.name if nc.partition_id_tensor else None
    in_names, out_names, out_avals = [], [], []
    for alloc in nc.m.functions[0].allocations:
        if not isinstance(alloc, mybir.MemoryLocationSet):
            continue
        name = alloc.memorylocations[0].name
        if alloc.kind == "ExternalInput":
            if name != pname:
                in_names.append(name)
        elif alloc.kind == "ExternalOutput":
            out_names.append(name)
            out_avals.append(jax.core.ShapedArray(
                tuple(alloc.tensor_shape), mybir.dt.np(alloc.dtype)))
    n_params, n_outs = len(in_names), len(out_names)
    bind_kw = dict(
        out_avals=tuple(out_avals),
        in_names=tuple(in_names + out_names + ([pname] if pname else [])),
        out_names=tuple(out_names),
        lowering_input_output_aliases=(),
        sim_require_finite=True,
        sim_require_nnan=True,
        nc=nc,
    )

    def _body(*args):
        ops = list(args)
        if pname:
            ops.append(partition_id_tensor())
        return tuple(_bass_exec_p.bind(*ops, **bind_kw))

    devices = jax.devices()[:8]
    mesh = Mesh(np.asarray(devices), ("core",))
    fn = jax.jit(
        shard_map(_body, mesh=mesh,
                  in_specs=(P("core"),) * (n_params + n_outs),
                  out_specs=(P("core"),) * n_outs, check_rep=False),
        keep_unused=True)
    zero_shapes = [(8 * a.shape[0], *a.shape[1:]) for a in out_avals]
    zero_dtypes = [a.dtype for a in out_avals]
    _CACHE["runner"] = (fn, in_names, out_names, zero_shapes, zero_dtypes, mesh)
    return _CACHE["runner"]


def kernel(x, wq, wdkv, wuk, wuv, wo, bo):
    x = np.asarray(x, np.float32)
    wq = np.asarray(wq, np.float32)
    wdkv = np.asarray(wdkv, np.float32)
    wuk = np.asarray(wuk, np.float32)
    wuv = np.asarray(wuv, np.float32)
    wo = np.asarray(wo, np.float32)
    bo = np.asarray(bo, np.float32)

    fn, in_names, out_names, zero_shapes, zero_dtypes, _ = _get_runner()
    in_maps = _make_in_maps(x, wq, wdkv, wuk, wuv, wo)
    args = [
        np.concatenate([np.asarray(in_maps[c][n]) for c in range(8)], axis=0)
        for n in in_names
    ] + [np.zeros(sh, dt) for sh, dt in zip(zero_shapes, zero_dtypes)]
    outs = fn(*args)
    outT = np.asarray(outs[out_names.index("outT")]).reshape(8, D, S)
    out = np.zeros((B, S, D), np.float32)
    for c in range(8):
        out[c // 4] += outT[c].T
    out += bo[None, None, :]
    return out
